# revision 4
# baseline (speedup 1.0000x reference)
"""GCN layer (symmetric-normalized, self-loops) on 8 Trainium2 NeuronCores.

out[d] = sum_{e:(s,d)} rsqrt(deg_s*deg_d) * (h_s @ W.T + b)

Factorization (linearity of the edge aggregation), rs = deg**-0.5:
  out[d] = rs_d * ( (sum_e rs_s * h_s) @ W.T + (sum_e rs_s) * b )

Device strategy (dst-sharded, SPMD over 8 cores, one instruction stream):
  - nodes are bin-packed into fixed windows of <=128 dst nodes; windows are
    processed in PAIRS so the per-chunk aggregation matmul has a 256-wide
    moving operand (float32r at 1 cycle/row on the PE).
  - per chunk of 128 edges: dma_gather 128 rows of h into SBUF partitions
    (indices are signed int16, so h is split into h0=h[:32768] / h1=rest),
    build a weighted one-hot S[e, dst_slot] = rs_src on the DVE (single
    dual-op tensor_scalar against an iota row), and accumulate
    P^T += X^T @ S on the PE (lhsT = gathered X chunk, rhs = S).
  - per window pair: P^T [feat, 256] sits in PSUM; for each window half,
    out_w = P^T.T @ W.T + wsum' (x) b accumulated in PSUM, then scaled by
    rs_d on the ACT copy out (per-partition scale), then DMA to DRAM.
Host (numpy) does only index/graph-metadata preparation: degree counts,
rs = deg**-0.5, wsum' = sum(rs_src) per dst, sorting/padding edges into the
fixed chunk schedule, and the inverse row permutation of the output.
"""

import sys

sys.path.insert(0, "/opt/trn_rl_repo")

import heapq

import numpy as np

N_NODES = 50000
D = 128
N_CORES = 8
H0 = 32768          # rows in first gather table (int16 index limit)
P = 128
KCH = 8             # chunks per dma_gather (1024 indices = SWDGE ring limit)

_COMPILED = {}


def _pack_windows(c0, c1, n_bins, cap0, cap1):
    """Assign each node to a window (bin) s.t. per-bin sums of c0/c1 stay
    under cap0/cap1 and <=128 nodes per bin.  Worst-fit greedy on total
    load, largest nodes first.  Returns win_of_node or None."""
    order = np.argsort(-(c0 + c1), kind="stable")
    heap = [(0, b) for b in range(n_bins)]
    heapq.heapify(heap)
    used0 = np.zeros(n_bins, np.int64)
    used1 = np.zeros(n_bins, np.int64)
    ncnt = np.zeros(n_bins, np.int64)
    win_of = np.full(N_NODES, -1, np.int32)
    for node in order:
        a = int(c0[node])
        b = int(c1[node])
        popped = []
        placed = False
        while heap:
            load, bidx = heapq.heappop(heap)
            if used0[bidx] + a <= cap0 and used1[bidx] + b <= cap1 and ncnt[bidx] < P:
                used0[bidx] += a
                used1[bidx] += b
                ncnt[bidx] += 1
                win_of[node] = bidx
                heapq.heappush(heap, (load + a + b, bidx))
                placed = True
                break
            if ncnt[bidx] < P:
                popped.append((load, bidx))
            # bins at node capacity are dropped permanently
        for it in popped:
            heapq.heappush(heap, it)
        if not placed:
            return None
    return win_of


def _wrap_idx(idx_flat):
    """dma_gather index layout: idx i -> partition i%16, col i//16,
    replicated 8x across the 128 partitions (one copy per gpsimd core)."""
    w = idx_flat.reshape(-1, 16).T
    return np.ascontiguousarray(np.tile(w, (8, 1)))


def _meta(dl, rw, nch):
    """meta layout [128, 2*nch]: col 2c = dst_slot of edge (chunk c, part e),
    col 2c+1 = rs_src."""
    dl2 = dl.reshape(nch, P)
    rw2 = rw.reshape(nch, P)
    m = np.empty((P, 2 * nch), np.float32)
    m[:, 0::2] = dl2.T
    m[:, 1::2] = rw2.T
    return np.ascontiguousarray(m)


def _preprocess(h, W, b, edges):
    src = np.concatenate([edges[0], np.arange(N_NODES, dtype=np.int64)]).astype(np.int64)
    dst = np.concatenate([edges[1], np.arange(N_NODES, dtype=np.int64)]).astype(np.int64)
    n_e = src.shape[0]

    deg = np.bincount(dst, minlength=N_NODES).astype(np.float32)
    rs = (deg ** -0.5).astype(np.float32)
    # wsum'[d] = sum_{e into d} rs_src  (bias coefficient; rs_d applied on-chip)
    wsum_full = np.bincount(dst, weights=rs[src].astype(np.float64),
                            minlength=N_NODES).astype(np.float32)

    half = (src >= H0).astype(np.int64)
    c0 = np.bincount(dst[half == 0], minlength=N_NODES)
    c1 = np.bincount(dst[half == 1], minlength=N_NODES)

    n_win = 52                              # windows per core (must be even)
    n_win_tot = n_win * N_CORES
    for T0, T1 in ((12, 7), (13, 8), (14, 9)):
        win_of = _pack_windows(c0, c1, n_win_tot, T0 * P, T1 * P)
        if win_of is not None:
            break
    assert win_of is not None, "window packing failed"

    slot_of = np.zeros(N_NODES, np.int32)
    win_nodes_count = np.zeros(n_win_tot, np.int32)
    order = np.argsort(win_of, kind="stable")
    for node in order:
        wg = win_of[node]
        slot_of[node] = win_nodes_count[wg]
        win_nodes_count[wg] += 1

    NC0, NC1 = n_win * T0, n_win * T1       # chunks per core per stream
    NG0 = -(-NC0 // KCH)                    # gathers per core per stream
    NG1 = -(-NC1 // KCH)

    # edge -> (window, half) group; position within group
    ew = win_of[dst].astype(np.int64)
    group = ew * 2 + half
    eorder = np.argsort(group, kind="stable")
    g_sorted = group[eorder]
    grp_start = np.searchsorted(g_sorted, np.arange(n_win_tot * 2), side="left")
    pos_in_grp = np.arange(n_e, dtype=np.int64) - grp_start[g_sorted]

    src_s = src[eorder]
    dst_s = dst[eorder]
    half_s = half[eorder]
    w_s = ew[eorder]
    core_s = w_s // n_win
    wloc_s = w_s % n_win

    chunk_in_win = pos_in_grp // P
    slot_in_chunk = pos_in_grp % P
    T_arr = np.where(half_s == 0, T0, T1)
    assert (chunk_in_win < T_arr).all()
    chunk_core = wloc_s * T_arr + chunk_in_win
    gpos = chunk_core * P + slot_in_chunk

    idx0 = np.zeros((N_CORES, NG0 * KCH * P), np.int16)
    idx1 = np.zeros((N_CORES, NG1 * KCH * P), np.int16)
    dl0 = np.zeros((N_CORES, NC0 * P), np.float32)
    rw0 = np.zeros((N_CORES, NC0 * P), np.float32)
    dl1 = np.zeros((N_CORES, NC1 * P), np.float32)
    rw1 = np.zeros((N_CORES, NC1 * P), np.float32)

    # dst slot within the WINDOW PAIR: + 128 for odd windows
    dstloc_s = slot_of[dst_s].astype(np.float32) + (wloc_s % 2).astype(np.float32) * P
    rsw_s = rs[src_s]

    m0 = half_s == 0
    m1 = ~m0
    idx0[core_s[m0], gpos[m0]] = src_s[m0].astype(np.int16)
    dl0[core_s[m0], gpos[m0]] = dstloc_s[m0]
    rw0[core_s[m0], gpos[m0]] = rsw_s[m0]
    idx1[core_s[m1], gpos[m1]] = (src_s[m1] - H0).astype(np.int16)
    dl1[core_s[m1], gpos[m1]] = dstloc_s[m1]
    rw1[core_s[m1], gpos[m1]] = rsw_s[m1]

    # per-core rswin [128, n_win] and wsum rows [1, n_win*128]
    rswin = np.ones((N_CORES, P, n_win), np.float32)
    wsumr = np.zeros((N_CORES, 1, n_win * P), np.float32)
    nodes_by_win_order = order  # nodes sorted by window
    wg_arr = win_of[nodes_by_win_order]
    slots_arr = slot_of[nodes_by_win_order]
    cores_arr = wg_arr // n_win
    wl_arr = wg_arr % n_win
    rswin[cores_arr, slots_arr, wl_arr] = rs[nodes_by_win_order]
    wsumr[cores_arr, 0, wl_arr * P + slots_arr] = wsum_full[nodes_by_win_order]

    h0 = np.ascontiguousarray(h[:H0])
    h1 = np.ascontiguousarray(h[H0:])
    Wt = np.ascontiguousarray(W.T)
    brow = np.ascontiguousarray(b.reshape(1, D))

    in_maps = []
    for c in range(N_CORES):
        in_maps.append({
            "h0": h0, "h1": h1,
            "idx0": np.ascontiguousarray(
                np.stack([_wrap_idx(idx0[c].reshape(NG0, KCH * P)[g])
                          for g in range(NG0)])),
            "idx1": np.ascontiguousarray(
                np.stack([_wrap_idx(idx1[c].reshape(NG1, KCH * P)[g])
                          for g in range(NG1)])),
            "meta0": _meta(dl0[c], rw0[c], NC0),
            "meta1": _meta(dl1[c], rw1[c], NC1),
            "rswin": np.ascontiguousarray(rswin[c]),
            "wsum": np.ascontiguousarray(wsumr[c]),
            "Wt": Wt, "b": brow,
        })

    out_perm_nodes = np.full((N_CORES, n_win * P), -1, np.int64)
    out_perm_nodes[cores_arr, wl_arr * P + slots_arr] = nodes_by_win_order

    geom = dict(T0=T0, T1=T1, n_win=n_win, NG0=NG0, NG1=NG1)
    return in_maps, out_perm_nodes, geom


def _build_nc(geom):
    import concourse.bacc as bacc
    import concourse.mybir as mybir
    import concourse.tile as tile

    T0, T1 = geom["T0"], geom["T1"]
    n_win = geom["n_win"]
    NG0, NG1 = geom["NG0"], geom["NG1"]
    NC0, NC1 = n_win * T0, n_win * T1
    f32, f32r, i16 = mybir.dt.float32, mybir.dt.float32r, mybir.dt.int16

    nc = bacc.Bacc("TRN2", target_bir_lowering=False, debug=False,
                   num_devices=N_CORES, num_swdge_queues=4)
    h0_d = nc.declare_dram_parameter("h0", [H0, D], f32r, isOutput=False)
    h1_d = nc.declare_dram_parameter("h1", [N_NODES - H0, D], f32r, isOutput=False)
    idx0_d = nc.declare_dram_parameter("idx0", [NG0, 128, KCH * 8], i16, isOutput=False)
    idx1_d = nc.declare_dram_parameter("idx1", [NG1, 128, KCH * 8], i16, isOutput=False)
    meta0_d = nc.declare_dram_parameter("meta0", [P, 2 * NC0], f32, isOutput=False)
    meta1_d = nc.declare_dram_parameter("meta1", [P, 2 * NC1], f32, isOutput=False)
    rswin_d = nc.declare_dram_parameter("rswin", [P, n_win], f32, isOutput=False)
    wsum_d = nc.declare_dram_parameter("wsum", [1, n_win * P], f32, isOutput=False)
    Wt_d = nc.declare_dram_parameter("Wt", [D, D], f32, isOutput=False)
    b_d = nc.declare_dram_parameter("b", [1, D], f32, isOutput=False)
    out_d = nc.declare_dram_parameter("out", [n_win * P, D], f32, isOutput=True)

    with tile.TileContext(nc) as tc:
        with (
            tc.tile_pool(name="const", bufs=1) as cpool,
            tc.tile_pool(name="xp0", bufs=4) as xp0,
            tc.tile_pool(name="xp1", bufs=4) as xp1,
            tc.tile_pool(name="ip", bufs=4) as ip,
            tc.tile_pool(name="sp", bufs=4) as sp,
            tc.tile_pool(name="wp", bufs=3) as wp,
            tc.tile_pool(name="ps", bufs=2, space="PSUM") as psA,
            tc.tile_pool(name="psO", bufs=2, space="PSUM") as psO,
        ):
            iota = cpool.tile([P, 2 * P], f32)
            nc.gpsimd.iota(iota[:], pattern=[[1, 2 * P]], base=0,
                           channel_multiplier=0,
                           allow_small_or_imprecise_dtypes=True)
            Wt_t = cpool.tile([D, D], f32)
            nc.sync.dma_start(out=Wt_t[:], in_=Wt_d[:])
            b_t = cpool.tile([1, D], f32)
            nc.sync.dma_start(out=b_t[:], in_=b_d[:])
            rswin_t = cpool.tile([P, n_win], f32)
            nc.sync.dma_start(out=rswin_t[:], in_=rswin_d[:])
            wsum_t = cpool.tile([1, n_win * P], f32)
            nc.sync.dma_start(out=wsum_t[:], in_=wsum_d[:])
            meta0_t = cpool.tile([P, 2 * NC0], f32)
            nc.sync.dma_start(out=meta0_t[:], in_=meta0_d[:])
            meta1_t = cpool.tile([P, 2 * NC1], f32)
            nc.sync.dma_start(out=meta1_t[:], in_=meta1_d[:])

            x0_tiles = [None] * NG0
            x1_tiles = [None] * NG1
            ng0_done = 0
            ng1_done = 0
            qn = 0

            def issue_g0():
                nonlocal ng0_done, qn
                g = ng0_done
                it = ip.tile([128, KCH * 8], i16, tag="i0")
                nc.sync.dma_start(out=it[:], in_=idx0_d[g])
                x = xp0.tile([P, KCH * P], f32r, tag="x0")
                nc.gpsimd.dma_gather(
                    out_ap=x[:].rearrange("p (c e) -> p c e", e=P),
                    in_ap=h0_d[:], idxs_ap=it[:],
                    num_idxs=KCH * P, num_idxs_reg=KCH * P, elem_size=P,
                    queue_num=qn % 4)
                qn += 1
                x0_tiles[g] = x
                ng0_done += 1

            def issue_g1():
                nonlocal ng1_done, qn
                g = ng1_done
                it = ip.tile([128, KCH * 8], i16, tag="i1")
                nc.sync.dma_start(out=it[:], in_=idx1_d[g])
                x = xp1.tile([P, KCH * P], f32r, tag="x1")
                nc.gpsimd.dma_gather(
                    out_ap=x[:].rearrange("p (c e) -> p c e", e=P),
                    in_ap=h1_d[:], idxs_ap=it[:],
                    num_idxs=KCH * P, num_idxs_reg=KCH * P, elem_size=P,
                    queue_num=qn % 4)
                qn += 1
                x1_tiles[g] = x
                ng1_done += 1

            def mm_chunk(pacc, c, meta_t, xt, start, stop):
                S_t = sp.tile([P, 2 * P], f32r, tag="S")
                nc.vector.tensor_scalar(
                    out=S_t[:], in0=iota[:],
                    scalar1=meta_t[:, 2 * c:2 * c + 1],
                    scalar2=meta_t[:, 2 * c + 1:2 * c + 2],
                    op0=mybir.AluOpType.is_equal,
                    op1=mybir.AluOpType.mult)
                nc.tensor.matmul(
                    out=pacc[:],
                    lhsT=xt,
                    rhs=S_t[:],
                    start=start, stop=stop)

            for pair in range(n_win // 2):
                # make sure every chunk this pair needs has been gathered
                need0 = (pair + 1) * 2 * T0
                need1 = (pair + 1) * 2 * T1
                while ng0_done * KCH < min(need0 + 2 * T0, NC0) and ng0_done < NG0:
                    issue_g0()
                while ng1_done * KCH < min(need1 + 2 * T1, NC1) and ng1_done < NG1:
                    issue_g1()

                pacc = psA.tile([P, 2 * P], f32, tag="pacc")
                n_mm = 2 * (T0 + T1)
                mi = 0
                for wl in (2 * pair, 2 * pair + 1):
                    for t in range(T0):
                        c = wl * T0 + t
                        xt = x0_tiles[c // KCH][:, (c % KCH) * P:(c % KCH + 1) * P]
                        mm_chunk(pacc, c, meta0_t, xt, mi == 0, mi == n_mm - 1)
                        mi += 1
                    for t in range(T1):
                        c = wl * T1 + t
                        xt = x1_tiles[c // KCH][:, (c % KCH) * P:(c % KCH + 1) * P]
                        mm_chunk(pacc, c, meta1_t, xt, mi == 0, mi == n_mm - 1)
                        mi += 1

                # tail: P^T [128 feat, 256 dst] in PSUM
                pt_sb = wp.tile([P, 2 * P], f32, tag="pt")
                nc.scalar.copy(out=pt_sb[:], in_=pacc[:])
                for hw in range(2):
                    w = 2 * pair + hw
                    out_ps = psO.tile([P, P], f32, tag="ops")
                    nc.tensor.matmul(out=out_ps[:],
                                     lhsT=pt_sb[:, hw * P:(hw + 1) * P],
                                     rhs=Wt_t[:], start=True, stop=False)
                    nc.tensor.matmul(out=out_ps[:],
                                     lhsT=wsum_t[:, w * P:(w + 1) * P],
                                     rhs=b_t[:], start=False, stop=True)
                    out_sb = wp.tile([P, P], f32, tag="osb")
                    nc.scalar.activation(out=out_sb[:], in_=out_ps[:],
                                         func=mybir.ActivationFunctionType.Copy,
                                         scale=rswin_t[:, w:w + 1])
                    nc.sync.dma_start(out=out_d[w * P:(w + 1) * P, :],
                                      in_=out_sb[:])

    nc.finalize()
    return nc


def _get_nc(geom):
    global mybir
    import concourse.mybir as mybir  # noqa: F401  (used in _build_nc closures)
    key = tuple(sorted(geom.items()))
    if key not in _COMPILED:
        _COMPILED[key] = _build_nc(geom)
    return _COMPILED[key]


def kernel(h, W, b, edges):
    from concourse.bass_utils import run_bass_kernel_spmd

    h = np.asarray(h, dtype=np.float32)
    W = np.asarray(W, dtype=np.float32)
    b = np.asarray(b, dtype=np.float32)
    edges = np.asarray(edges)

    in_maps, out_perm_nodes, geom = _preprocess(h, W, b, edges)
    nc = _get_nc(geom)
    res = run_bass_kernel_spmd(nc, in_maps, list(range(N_CORES)))

    out = np.zeros((N_NODES, D), np.float32)
    for c in range(N_CORES):
        rows = out_perm_nodes[c]
        valid = rows >= 0
        out[rows[valid]] = res.results[c]["out"][valid]
    return out
